# revision 41
# baseline (speedup 1.0000x reference)
"""GQA attention (RoPE + causal softmax + out-proj) on 8 TRN2 cores.

Sharding: one core per (batch b, kv-head-group g): 2 batches x 4 kv groups = 8
cores. Each core computes its group's 4 query heads end to end, including the
partial output projection through its 256 rows of wo; the host sums the 4
partial projections per batch (fp16 partials, f32 host sum).

v3: measured on HW, fp8 DoubleRow matmuls trip the PE's half-rate power
throttle when used broadly (util limit 53%, DR avg 589ns vs fp16 483ns), so
fp16 is used for all high-volume matmuls (projections, scores, out-proj,
baseline-style tile_position packing) and fp8 DoubleRow is kept ONLY where it
halves the instruction count: PV over 256-row k superblocks, with the exp'd
probabilities written by ACT directly into the DoubleRow rhs layout
p8 [128p(k), 2(kb parity), 2(head of pair), 512] -- zero data movement.
Structural wins vs the fp16 baseline:
  - no mask matmuls: causality via partial-width exp + gpsimd 0/1-triangle
    multiply + gpsimd memset of the dead region.
  - PV lhsT carries an all-ones column, so each head's [65+, 512] psum tile
    accumulates its softmax denominator in row 64 (DoubleRow dst must start
    at partition 0); one reciprocal per chunk + SELP broadcast matmuls.
  - rows 0:512 (chunk 0) run fully in fp16 (P and V): tiny softmax support
    there means fp8 P/V noise does not average out and would blow the
    max-norm error; later rows have large support and tolerate fp8.
  - software pipeline: projection matmuls for chunk c+2 are emitted between
    attention(c) and the norm chain so PE has work while DVE runs the
    reciprocal; rope/transposes follow after.
  - fp16 output tensor (host sums partials in f32): halves the output DMA.
"""

import os
import sys
import types

import numpy as np
import ml_dtypes


def _ensure_axon_hooks_shim():
    """The agent image's antenv package lacks the axon_hooks submodule that
    concourse's trace path imports; install a stub so trace requests degrade
    to no-trace instead of crashing (a real hook can be set into the stub)."""
    try:
        import antenv.axon_hooks  # noqa: F401

        return
    except ImportError:
        pass
    try:
        import antenv
    except ImportError:
        return
    mod = types.ModuleType("antenv.axon_hooks")
    mod._AXON_NTFF_PROFILE_HOOK = None

    def get_axon_ntff_profile_hook():
        return mod._AXON_NTFF_PROFILE_HOOK

    def set_axon_ntff_profile_hook(hook):
        mod._AXON_NTFF_PROFILE_HOOK = hook

    mod.get_axon_ntff_profile_hook = get_axon_ntff_profile_hook
    mod.set_axon_ntff_profile_hook = set_axon_ntff_profile_hook
    sys.modules["antenv.axon_hooks"] = mod
    antenv.axon_hooks = mod


_ensure_axon_hooks_shim()

import concourse.bass as bass
import concourse.bacc as bacc
import concourse.mybir as mybir
import concourse.tile as tile
from concourse.bass_utils import run_bass_kernel_spmd

F32 = mybir.dt.float32
F16 = mybir.dt.float16
F8 = mybir.dt.float8e4
AF = mybir.ActivationFunctionType
OP = mybir.AluOpType
DR = mybir.MatmulPerfMode.DoubleRow
FP8T = ml_dtypes.float8_e4m3

B, DIM = 2, 1024
NH, NKV, HD = 16, 4, 64
S_FULL = 2048
SC = 512  # q chunk width
KT = DIM // 128  # 8 k-tiles over the model dim


def build_nc(S=S_FULL, n_cores=8):
    NCH = S // SC
    NSB = S // 256  # k superblocks (256 rows each)

    nc = bacc.Bacc(
        "TRN2", target_bir_lowering=False, debug=False, num_devices=n_cores
    )
    xt16d = nc.dram_tensor("xt16", [128, KT, S], F16, kind="ExternalInput").ap()
    wq16d = nc.dram_tensor("wq16", [128, KT, 256], F16, kind="ExternalInput").ap()
    wkv16d = nc.dram_tensor("wkv16", [128, KT, 128], F16, kind="ExternalInput").ap()
    wod = nc.dram_tensor("wo", [128, 2, DIM], F16, kind="ExternalInput").ap()
    cosr = nc.dram_tensor("cosr", [128, S], F16, kind="ExternalInput").ap()
    sinr = nc.dram_tensor("sinr", [128, S], F16, kind="ExternalInput").ap()
    mask2d = nc.dram_tensor("mask2", [128, 2, 128], F8, kind="ExternalInput").ap()
    mask16d = nc.dram_tensor("mask16", [128, 2, 128], F16, kind="ExternalInput").ap()
    ident16d = nc.dram_tensor("ident16", [64, 64], F16, kind="ExternalInput").ap()
    selpd = nc.dram_tensor("selp", [128, 2, 128], F16, kind="ExternalInput").ap()
    out = nc.dram_tensor("out", [S, DIM], F16, kind="ExternalOutput").ap()

    with tile.TileContext(nc) as tc:
        with (
            tc.tile_pool(name="const", bufs=1) as cp,
            tc.tile_pool(name="sc", bufs=2, space="PSUM") as scp_pool,
            tc.tile_pool(name="pv", bufs=4, space="PSUM") as pvp,
            tc.tile_pool(name="rt", bufs=2) as rt,
            tc.tile_pool(name="qr", bufs=3) as qr,
            tc.tile_pool(name="pp", bufs=2) as pp,
            tc.tile_pool(name="np_", bufs=2) as npo,
            tc.tile_pool(name="op", bufs=3) as op_pool,
        ):
            COS = cp.tile([128, S], F16, tag="COS")
            SIN = cp.tile([128, S], F16, tag="SIN")
            WO = cp.tile([128, 2, DIM], F16, tag="WO")
            MASK2 = cp.tile([128, 2, 128], F8, tag="MASK2")
            MASK16 = cp.tile([128, 2, 128], F16, tag="MASK16")
            SELP = cp.tile([128, 2, 128], F16, tag="SELP")
            XT16 = cp.tile([128, KT, S], F16, tag="XT16")
            WQ16 = cp.tile([128, KT, 256], F16, tag="WQ16")
            WKV16 = cp.tile([128, KT, 128], F16, tag="WKV16")
            IDENT16 = cp.tile([64, 64], F16, tag="IDENT16")
            KA4 = cp.tile([128, S], F16, tag="KA4")  # [kre;kim] x2 row-dup
            VT16 = cp.tile([64, S], F16, tag="VT16")
            VAUG8 = cp.tile([128, NSB, 2, 128], F8, tag="VAUG8")
            VAUG16 = cp.tile([128, 4, 65], F16, tag="VAUG16")
            AT0 = cp.tile([128, S], F16, tag="AT0")
            AT1 = cp.tile([128, S], F16, tag="AT1")

            # startup DMAs: each dma_start costs ~0.6us of ISSUE time on
            # its queue, so spread across sync/scalar/vector queues and order
            # by first use (proj(0) needs WQ16 + XT16 head immediately)
            nc.scalar.dma_start(WQ16[:], wq16d)
            nc.scalar.dma_start(WKV16[:], wkv16d)
            nc.scalar.dma_start(MASK16[:], mask16d)
            nc.scalar.dma_start(MASK2[:], mask2d)
            nc.scalar.dma_start(COS[:], cosr)
            nc.scalar.dma_start(SIN[:], sinr)
            nc.scalar.dma_start(IDENT16[:], ident16d)
            nc.scalar.dma_start(SELP[:], selpd)
            for kt in range(KT):
                nc.sync.dma_start(XT16[:, kt, 0:SC], xt16d[:, kt, 0:SC])
            if S > SC:
                for kt in range(KT):
                    nc.sync.dma_start(XT16[:, kt, SC:], xt16d[:, kt, SC:])
            nc.sync.dma_start(WO[:], wod)
            # ones column for the PV denominators (DoubleRow dst must start
            # at partition 0, so dens ride along as output row 64)
            nc.vector.memset(VAUG8[:, :, :, 64:128], 0.0)
            nc.vector.memset(VAUG8[:, :, :, 64:65], 1.0)
            nc.vector.memset(VAUG16[:, :, 64:65], 1.0)

            def proj_mm(qc):
                """Projection matmuls + psum->sbuf copies for chunk qc.
                Emitted early so PE has work while the norm chain runs."""
                sl = slice(qc * SC, (qc + 1) * SC)
                q0t = scp_pool.tile([128, 2, SC], F32, tag="sc", name="q0t")
                q0 = q0t[:, 0, :]
                for kt in range(KT):
                    nc.tensor.matmul(
                        q0, WQ16[:, kt, 0:128], XT16[:, kt, sl],
                        start=(kt == 0), stop=(kt == KT - 1),
                    )
                q0s = rt.tile([128, SC], F16, tag="q0s")
                nc.scalar.copy(q0s[:], q0)
                q1t = scp_pool.tile([128, 2, SC], F32, tag="sc", name="q1t")
                q1 = q1t[:, 0, :]
                for kt in range(KT):
                    nc.tensor.matmul(
                        q1, WQ16[:, kt, 128:256], XT16[:, kt, sl],
                        start=(kt == 0), stop=(kt == KT - 1),
                    )
                q1s = rt.tile([128, SC], F16, tag="q1s")
                nc.scalar.copy(q1s[:], q1)
                kvt = scp_pool.tile([128, 2, SC], F32, tag="sc", name="kvt")
                kv = kvt[:, 0, :]
                for kt in range(KT):
                    nc.tensor.matmul(
                        kv, WKV16[:, kt, :], XT16[:, kt, sl],
                        start=(kt == 0), stop=(kt == KT - 1),
                    )
                kvs = rt.tile([128, SC], F16, tag="kvs")
                nc.scalar.copy(kvs[:], kv)
                return q0s, q1s, kvs

            def rope_part(qc, q0s, q1s, kvs):
                """RoPE combines + V transposes for chunk qc; returns the
                fp16 REIM pair tiles (rows [re_h;im_h] per head)."""
                sl = slice(qc * SC, (qc + 1) * SC)
                prec = qc == 0
                # v first: f16 transpose (PE) can start as soon as kvs lands
                nc.vector.tensor_copy(VT16[0:64, sl], kvs[64:128, :])
                for i4 in range(4):
                    kbg = 4 * qc + i4
                    sbi, ip = kbg // 2, kbg % 2
                    vp = scp_pool.tile([128, 64], F16, tag="sc", name="vp")
                    nc.tensor.transpose(
                        vp[:], VT16[0:64, kbg * 128 : (kbg + 1) * 128], IDENT16[:]
                    )
                    nc.vector.tensor_copy(VAUG8[:, sbi, ip, 0:64], vp[:])
                    if prec:
                        nc.vector.tensor_copy(VAUG16[:, i4, 0:64], vp[:])
                t1 = rt.tile([128, SC], F16, tag="t1")
                t2 = rt.tile([128, SC], F16, tag="t2")
                t3 = rt.tile([128, SC], F16, tag="t3")
                t4 = rt.tile([128, SC], F16, tag="t4")
                nc.vector.tensor_tensor(t1[:], q0s[:], COS[:, sl], OP.mult)
                nc.vector.tensor_tensor(t2[:], q1s[:], SIN[:, sl], OP.mult)
                nc.vector.tensor_tensor(t3[:], q0s[:], SIN[:, sl], OP.mult)
                nc.vector.tensor_tensor(t4[:], q1s[:], COS[:, sl], OP.mult)
                QF0 = qr.tile([128, SC], F16, tag="qf0")
                QF1 = qr.tile([128, SC], F16, tag="qf1")
                for h in range(4):
                    QF, half = (QF0, QF1)[h // 2], h % 2
                    rq = slice(32 * h, 32 * h + 32)
                    nc.vector.tensor_tensor(
                        QF[64 * half : 64 * half + 32, :],
                        t1[rq, :], t2[rq, :], OP.subtract,
                    )
                    nc.vector.tensor_tensor(
                        QF[64 * half + 32 : 64 * half + 64, :],
                        t3[rq, :], t4[rq, :], OP.add,
                    )

                # k rope into KA4 rows 0:64, duplicated to 64:128
                u1 = rt.tile([32, SC], F16, tag="u1")
                u2 = rt.tile([32, SC], F16, tag="u2")
                u3 = rt.tile([32, SC], F16, tag="u3")
                u4 = rt.tile([32, SC], F16, tag="u4")
                nc.vector.tensor_tensor(u1[:], kvs[0:32, :], COS[0:32, sl], OP.mult)
                nc.vector.tensor_tensor(u2[:], kvs[32:64, :], SIN[32:64, sl], OP.mult)
                nc.vector.tensor_tensor(KA4[0:32, sl], u1[:], u2[:], OP.subtract)
                nc.vector.tensor_tensor(u3[:], kvs[0:32, :], SIN[0:32, sl], OP.mult)
                nc.vector.tensor_tensor(u4[:], kvs[32:64, :], COS[32:64, sl], OP.mult)
                nc.vector.tensor_tensor(KA4[32:64, sl], u3[:], u4[:], OP.add)
                nc.sync.dma_start(KA4[64:128, sl], KA4[0:64, sl])

                return QF0, QF1

            def att_scores(qc, QF, sbi, p8s):
                """scores+exp+mask for superblock sbi of chunk qc; appends the
                p8 tile pair (one per head-pair) to p8s."""
                prec = qc == 0
                pr_tiles = [
                    pp.tile(
                        [128, 2, 2, SC], F16 if prec else F8,
                        tag=f"p16_{pr}" if prec else f"p8_{pr}",
                        name=f"p8_{pr}", bufs=6,
                    )
                    for pr in range(2)
                ]
                p8s.append(pr_tiles)
                for ip in range(2):
                    kb = 2 * sbi + ip
                    kre = kb - 4 * qc  # >= 0: diagonal block index
                    qlo = 128 * kre if kre > 0 else 0
                    ksl = slice(kb * 128, (kb + 1) * 128)
                    for pr in range(2):
                        scps = scp_pool.tile([128, 2, SC], F32, tag="sc")
                        for j in range(2):
                            rs = slice(64 * j, 64 * j + 64)
                            nc.tensor.matmul(
                                scps[:, j, qlo:SC],
                                KA4[rs, ksl],
                                QF[pr][rs, qlo:SC],
                                start=True, stop=True,
                                tile_position=(64 * j, 0),
                            )
                        p8 = pr_tiles[pr]
                        nc.scalar.activation(
                            p8[:, ip, :, qlo:SC], scps[:, :, qlo:SC],
                            AF.Exp, scale=0.125,
                        )
                        if kre >= 0:
                            if qlo > 0:
                                nc.vector.memset(p8[:, ip, :, 0:qlo], 0.0)
                            msl = slice(128 * kre, 128 * kre + 128)
                            nc.vector.tensor_tensor(
                                p8[:, ip, :, msl], p8[:, ip, :, msl],
                                MASK16[:] if prec else MASK2[:], OP.mult,
                            )

            def att_pv(qc, sbi, pr_tiles, ots):
                nkb = 4 * qc + 4
                nsb = nkb // 2
                prec = qc == 0
                for pr in range(2):
                    for j in range(2):
                        h = 2 * pr + j
                        if prec:
                            for ip in range(2):
                                kb = 2 * sbi + ip
                                nc.tensor.matmul(
                                    ots[h][0:65, :],
                                    VAUG16[:, kb, :],
                                    pr_tiles[pr][:, ip, j, :],
                                    start=(kb == 0), stop=(kb == nkb - 1),
                                )
                        else:
                            nc.tensor.matmul(
                                ots[h][:, :],
                                VAUG8[:, sbi, :, 0:128],
                                pr_tiles[pr][:, :, j, :],
                                start=(sbi == 0), stop=(sbi == nsb - 1),
                                perf_mode=DR,
                            )

            def norm_pre(qc, ots):
                """denominator gather + reciprocal; emit FIRST so the scalar
                copies sit ahead of the next chunk's exps in the ACT queue."""
                den_sb = npo.tile([128, SC], F32, tag="den_sb")
                nc.vector.memset(den_sb[:], 1.0)
                for h in range(4):
                    nc.vector.tensor_copy(
                        den_sb[32 * h : 32 * h + 1, :], ots[h][64:65, :]
                    )
                rec = npo.tile([128, SC], F16, tag="rec")
                with nc.allow_low_precision(reason="fp16 softmax denominators"):
                    nc.vector.reciprocal(rec[:], den_sb[:])
                return rec

            def norm_post(qc, ots, rec):
                qsl = slice(qc * SC, (qc + 1) * SC)
                for pr in range(2):
                    rbct = scp_pool.tile([128, 2, SC], F32, tag="sc", name=f"rbc{pr}")
                    rbc = rbct[:, 0, :]
                    nc.tensor.matmul(
                        rbc, SELP[:, pr, :], rec[:], start=True, stop=True
                    )
                    rbc_sb = npo.tile([128, SC], F16, tag="rbc_sb")
                    nc.vector.tensor_copy(rbc_sb[:], rbc)
                    att = (AT0, AT1)[pr]
                    for j in range(2):
                        rs = slice(64 * j, 64 * j + 64)
                        nc.vector.tensor_tensor(
                            att[rs, qsl], ots[2 * pr + j][0:64, :],
                            rbc_sb[rs, :], OP.mult,
                        )

            def oproj_one(qc, k):
                """k-th of the 8 out-proj blocks for chunk qc."""
                sb_i = 4 * qc + k // 2
                ec = k % 2
                ssl = slice(sb_i * 128, (sb_i + 1) * 128)
                esl = slice(ec * 512, (ec + 1) * 512)
                o_ps = scp_pool.tile([128, 2, SC], F32, tag="sc", name="o_ps")
                for t in range(2):
                    att = (AT0, AT1)[t]
                    nc.tensor.matmul(
                        o_ps[:, 0, :], att[:, ssl], WO[:, t, esl],
                        start=(t == 0), stop=(t == 1),
                    )
                ost = op_pool.tile([128, 512], F16, tag="ost")
                if k % 2 == 0:
                    nc.vector.tensor_copy(ost[:], o_ps[:, 0, :])
                    nc.sync.dma_start(out[ssl, esl], ost[:])
                else:
                    nc.scalar.copy(ost[:], o_ps[:, 0, :])
                    nc.scalar.dma_start(out[ssl, esl], ost[:])

            qfs = {}
            st0 = proj_mm(0)
            # HAM warm-up + startup-gap filler: dummy matmuls into a scratch
            # pv bank while DVE/scalar run chunk 0's rope chain
            warm = pvp.tile([128, SC], F32, tag="pv", name="warm")
            for _ in range(10):
                nc.tensor.matmul(
                    warm[:], WQ16[:, 0, 0:128], XT16[:, 0, 0:SC],
                    start=True, stop=True,
                )
            qfs[0] = rope_part(0, *st0)
            if NCH > 1:
                qfs[1] = rope_part(1, *proj_mm(1))
            prev = None  # (qc, ots) awaiting norm+oproj
            for qc in range(NCH):
                QF = qfs.pop(qc)
                nsb = (4 * qc + 4) // 2
                ots = [
                    pvp.tile([128, SC], F32, tag="pv", name=f"ot{qc}_{h}")
                    for h in range(4)
                ]
                p8s = []
                rec = norm_pre(*prev) if prev is not None else None
                # scores for the first superblocks keep PE+ACT busy while the
                # previous chunk's norm chain (DVE recip) runs
                pref = min(nsb, 5)
                for sbi in range(pref):
                    att_scores(qc, QF, sbi, p8s)
                if prev is not None:
                    norm_post(*prev, rec)
                for sbi in range(pref):
                    att_pv(qc, sbi, p8s[sbi], ots)
                # remaining superblocks, with the previous chunk's out-proj
                # spread between them so PE never idles through a MID window
                oleft = list(range(8)) if prev is not None else []
                rem = max(nsb - pref, 1)
                per = (len(oleft) + rem - 1) // rem if oleft else 0
                for sbi in range(pref, nsb):
                    att_scores(qc, QF, sbi, p8s)
                    for _ in range(per):
                        if oleft:
                            oproj_one(prev[0], oleft.pop(0))
                    att_pv(qc, sbi, p8s[sbi], ots)
                while oleft:
                    oproj_one(prev[0], oleft.pop(0))
                st = proj_mm(qc + 2) if qc + 2 < NCH else None
                prev = (qc, ots)
                if st is not None:
                    qfs[qc + 2] = rope_part(qc + 2, *st)
            norm_post(*prev, norm_pre(*prev))
            for k in range(8):
                oproj_one(prev[0], k)

    nc.compile()
    return nc


# host-side column permutations: all rope-even dims first, then all odds
_PERM256 = np.array(
    [64 * h + 2 * i for h in range(4) for i in range(32)]
    + [64 * h + 2 * i + 1 for h in range(4) for i in range(32)]
)
_PERM64 = np.array([2 * i for i in range(32)] + [2 * i + 1 for i in range(32)])

_cache = {}


def _fp8(a):
    return np.clip(a, -240.0, 240.0).astype(FP8T)


def make_in_maps(x, cos, sin, wq, wk, wv, wo, n_groups=4):
    S = x.shape[1]
    cos_r = np.ascontiguousarray(np.tile(cos.T, (4, 1)), dtype=np.float16)
    sin_r = np.ascontiguousarray(np.tile(sin.T, (4, 1)), dtype=np.float16)
    # scores_T[k, q] valid iff q >= k within the diagonal 128-block
    pidx, qidx = np.meshgrid(np.arange(128), np.arange(128), indexing="ij")
    tri = (qidx >= pidx).astype(np.float32)
    mask2 = _fp8(np.broadcast_to(tri[:, None, :], (128, 2, 128)).copy())
    mask16 = np.ascontiguousarray(
        np.broadcast_to(tri[:, None, :], (128, 2, 128)).astype(np.float16)
    )
    ident16 = np.eye(64, dtype=np.float16)
    selp = np.zeros((128, 2, 128), dtype=np.float16)
    selp[0, 0, 0:64] = 1.0
    selp[32, 0, 64:128] = 1.0
    selp[64, 1, 0:64] = 1.0
    selp[96, 1, 64:128] = 1.0
    xt16s = []
    for b in range(x.shape[0]):
        xT = x[b].T.astype(np.float16)  # [1024, S]
        xt16s.append(
            np.ascontiguousarray(xT.reshape(KT, 128, S).transpose(1, 0, 2))
        )
    in_maps = []
    for c in range(x.shape[0] * n_groups):
        b, g = divmod(c, n_groups)
        wq_c = wq[:, 256 * g + _PERM256].astype(np.float16)
        wk_c = wk[:, 64 * g + _PERM64]
        wv_c = wv[:, 64 * g : 64 * (g + 1)]
        wkv_c = np.concatenate([wk_c, wv_c], axis=1).astype(np.float16)
        wo_c = np.ascontiguousarray(
            wo[256 * g : 256 * (g + 1), :].astype(np.float16)
            .reshape(2, 128, DIM).transpose(1, 0, 2)
        )
        in_maps.append(
            {
                "xt16": xt16s[b],
                "wq16": np.ascontiguousarray(
                    wq_c.reshape(KT, 128, 256).transpose(1, 0, 2)
                ),
                "wkv16": np.ascontiguousarray(
                    wkv_c.reshape(KT, 128, 128).transpose(1, 0, 2)
                ),
                "wo": wo_c,
                "cosr": cos_r,
                "sinr": sin_r,
                "mask2": mask2,
                "mask16": mask16,
                "ident16": ident16,
                "selp": selp,
            }
        )
    return in_maps


def kernel(x, cos, sin, mask, wq, wk, wv, wo):
    x = np.asarray(x, dtype=np.float32)
    cos = np.asarray(cos, dtype=np.float32)
    sin = np.asarray(sin, dtype=np.float32)
    wq = np.asarray(wq, dtype=np.float32)
    wk = np.asarray(wk, dtype=np.float32)
    wv = np.asarray(wv, dtype=np.float32)
    wo = np.asarray(wo, dtype=np.float32)

    if "nc" not in _cache:
        _cache["nc"] = build_nc(S=x.shape[1], n_cores=8)
    nc = _cache["nc"]
    in_maps = make_in_maps(x, cos, sin, wq, wk, wv, wo)
    res = run_bass_kernel_spmd(nc, in_maps, list(range(8)))
    _cache["last"] = res
    outs = [np.asarray(r["out"], dtype=np.float32) for r in res.results]
    final = np.stack(
        [outs[0] + outs[1] + outs[2] + outs[3], outs[4] + outs[5] + outs[6] + outs[7]],
        axis=0,
    )
    return final.astype(np.float32)


# revision 42
# speedup vs baseline: 1.2148x; 1.2148x over previous
"""GQA attention (RoPE + causal softmax + out-proj) on 8 TRN2 cores.

Sharding: one core per (batch b, kv-head-group g): 2 batches x 4 kv groups = 8
cores. Each core computes its group's 4 query heads end to end, including the
partial output projection through its 256 rows of wo; the host sums the 4
partial projections per batch (fp16 partials, f32 host sum).

v3: measured on HW, fp8 DoubleRow matmuls trip the PE's half-rate power
throttle when used broadly (util limit 53%, DR avg 589ns vs fp16 483ns), so
fp16 is used for all high-volume matmuls (projections, scores, out-proj,
baseline-style tile_position packing) and fp8 DoubleRow is kept ONLY where it
halves the instruction count: PV over 256-row k superblocks, with the exp'd
probabilities written by ACT directly into the DoubleRow rhs layout
p8 [128p(k), 2(kb parity), 2(head of pair), 512] -- zero data movement.
Structural wins vs the fp16 baseline:
  - no mask matmuls: causality via partial-width exp + gpsimd 0/1-triangle
    multiply + gpsimd memset of the dead region.
  - PV lhsT carries an all-ones column, so each head's [65+, 512] psum tile
    accumulates its softmax denominator in row 64 (DoubleRow dst must start
    at partition 0); one reciprocal per chunk + SELP broadcast matmuls.
  - rows 0:512 (chunk 0) run fully in fp16 (P and V): tiny softmax support
    there means fp8 P/V noise does not average out and would blow the
    max-norm error; later rows have large support and tolerate fp8.
  - software pipeline: projection matmuls for chunk c+2 are emitted between
    attention(c) and the norm chain so PE has work while DVE runs the
    reciprocal; rope/transposes follow after.
  - fp16 output tensor (host sums partials in f32): halves the output DMA.
"""

import os
import sys
import types

import numpy as np
import ml_dtypes


def _ensure_axon_hooks_shim():
    """The agent image's antenv package lacks the axon_hooks submodule that
    concourse's trace path imports; install a stub so trace requests degrade
    to no-trace instead of crashing (a real hook can be set into the stub)."""
    try:
        import antenv.axon_hooks  # noqa: F401

        return
    except ImportError:
        pass
    try:
        import antenv
    except ImportError:
        return
    mod = types.ModuleType("antenv.axon_hooks")
    mod._AXON_NTFF_PROFILE_HOOK = None

    def get_axon_ntff_profile_hook():
        return mod._AXON_NTFF_PROFILE_HOOK

    def set_axon_ntff_profile_hook(hook):
        mod._AXON_NTFF_PROFILE_HOOK = hook

    mod.get_axon_ntff_profile_hook = get_axon_ntff_profile_hook
    mod.set_axon_ntff_profile_hook = set_axon_ntff_profile_hook
    sys.modules["antenv.axon_hooks"] = mod
    antenv.axon_hooks = mod


_ensure_axon_hooks_shim()

import concourse.bass as bass
import concourse.bacc as bacc
import concourse.mybir as mybir
import concourse.tile as tile
from concourse.bass_utils import run_bass_kernel_spmd

F32 = mybir.dt.float32
F16 = mybir.dt.float16
F8 = mybir.dt.float8e4
AF = mybir.ActivationFunctionType
OP = mybir.AluOpType
DR = mybir.MatmulPerfMode.DoubleRow
FP8T = ml_dtypes.float8_e4m3

B, DIM = 2, 1024
NH, NKV, HD = 16, 4, 64
S_FULL = 2048
SC = 512  # q chunk width
KT = DIM // 128  # 8 k-tiles over the model dim


def build_nc(S=S_FULL, n_cores=8):
    NCH = S // SC
    NSB = S // 256  # k superblocks (256 rows each)

    nc = bacc.Bacc(
        "TRN2", target_bir_lowering=False, debug=False, num_devices=n_cores
    )
    xt16d = nc.dram_tensor("xt16", [128, KT, S], F16, kind="ExternalInput").ap()
    wq16d = nc.dram_tensor("wq16", [128, KT, 256], F16, kind="ExternalInput").ap()
    wkv16d = nc.dram_tensor("wkv16", [128, KT, 128], F16, kind="ExternalInput").ap()
    wod = nc.dram_tensor("wo", [128, 2, DIM], F16, kind="ExternalInput").ap()
    cosr = nc.dram_tensor("cosr", [128, S], F16, kind="ExternalInput").ap()
    sinr = nc.dram_tensor("sinr", [128, S], F16, kind="ExternalInput").ap()
    mask2d = nc.dram_tensor("mask2", [128, 2, 128], F8, kind="ExternalInput").ap()
    mask16d = nc.dram_tensor("mask16", [128, 2, 128], F16, kind="ExternalInput").ap()
    ident16d = nc.dram_tensor("ident16", [64, 64], F16, kind="ExternalInput").ap()
    selpd = nc.dram_tensor("selp", [128, 2, 128], F16, kind="ExternalInput").ap()
    out = nc.dram_tensor("out", [S, DIM], F16, kind="ExternalOutput").ap()

    with tile.TileContext(nc) as tc:
        with (
            tc.tile_pool(name="const", bufs=1) as cp,
            tc.tile_pool(name="sc", bufs=2, space="PSUM") as scp_pool,
            tc.tile_pool(name="pv", bufs=4, space="PSUM") as pvp,
            tc.tile_pool(name="rt", bufs=2) as rt,
            tc.tile_pool(name="qr", bufs=3) as qr,
            tc.tile_pool(name="pp", bufs=2) as pp,
            tc.tile_pool(name="np_", bufs=2) as npo,
            tc.tile_pool(name="op", bufs=3) as op_pool,
        ):
            COS = cp.tile([128, S], F16, tag="COS")
            SIN = cp.tile([128, S], F16, tag="SIN")
            WO = cp.tile([128, 2, DIM], F16, tag="WO")
            MASK2 = cp.tile([128, 2, 128], F8, tag="MASK2")
            MASK16 = cp.tile([128, 2, 128], F16, tag="MASK16")
            SELP = cp.tile([128, 2, 128], F16, tag="SELP")
            XT16 = cp.tile([128, KT, S], F16, tag="XT16")
            WQ16 = cp.tile([128, KT, 256], F16, tag="WQ16")
            WKV16 = cp.tile([128, KT, 128], F16, tag="WKV16")
            IDENT16 = cp.tile([64, 64], F16, tag="IDENT16")
            KA4 = cp.tile([128, S], F16, tag="KA4")  # [kre;kim] x2 row-dup
            VT16 = cp.tile([64, S], F16, tag="VT16")
            VAUG8 = cp.tile([128, NSB, 2, 128], F8, tag="VAUG8")
            VAUG16 = cp.tile([128, 4, 65], F16, tag="VAUG16")
            AT0 = cp.tile([128, S], F16, tag="AT0")
            AT1 = cp.tile([128, S], F16, tag="AT1")

            # startup DMAs: each dma_start costs ~0.6us of ISSUE time on
            # its queue, so spread across sync/scalar/vector queues and order
            # by first use (proj(0) needs WQ16 + XT16 head immediately)
            nc.scalar.dma_start(WQ16[:], wq16d)
            nc.scalar.dma_start(WKV16[:], wkv16d)
            nc.scalar.dma_start(MASK16[:], mask16d)
            nc.scalar.dma_start(MASK2[:], mask2d)
            nc.scalar.dma_start(COS[:], cosr)
            nc.scalar.dma_start(SIN[:], sinr)
            nc.scalar.dma_start(IDENT16[:], ident16d)
            nc.scalar.dma_start(SELP[:], selpd)
            for kt in range(KT):
                nc.sync.dma_start(XT16[:, kt, 0:SC], xt16d[:, kt, 0:SC])
            if S > SC:
                for kt in range(KT):
                    nc.sync.dma_start(XT16[:, kt, SC:], xt16d[:, kt, SC:])
            nc.sync.dma_start(WO[:], wod)
            # ones column for the PV denominators (DoubleRow dst must start
            # at partition 0, so dens ride along as output row 64)
            nc.vector.memset(VAUG8[:, :, :, 64:128], 0.0)
            nc.vector.memset(VAUG8[:, :, :, 64:65], 1.0)
            nc.vector.memset(VAUG16[:, :, 64:65], 1.0)

            def proj_mm(qc):
                """Projection matmuls + psum->sbuf copies for chunk qc.
                Emitted early so PE has work while the norm chain runs."""
                sl = slice(qc * SC, (qc + 1) * SC)
                q0t = scp_pool.tile([128, 2, SC], F32, tag="sc", name="q0t")
                q0 = q0t[:, 0, :]
                for kt in range(KT):
                    nc.tensor.matmul(
                        q0, WQ16[:, kt, 0:128], XT16[:, kt, sl],
                        start=(kt == 0), stop=(kt == KT - 1),
                    )
                q0s = rt.tile([128, SC], F16, tag="q0s")
                nc.scalar.copy(q0s[:], q0)
                q1t = scp_pool.tile([128, 2, SC], F32, tag="sc", name="q1t")
                q1 = q1t[:, 0, :]
                for kt in range(KT):
                    nc.tensor.matmul(
                        q1, WQ16[:, kt, 128:256], XT16[:, kt, sl],
                        start=(kt == 0), stop=(kt == KT - 1),
                    )
                q1s = rt.tile([128, SC], F16, tag="q1s")
                nc.scalar.copy(q1s[:], q1)
                kvt = scp_pool.tile([128, 2, SC], F32, tag="sc", name="kvt")
                kv = kvt[:, 0, :]
                for kt in range(KT):
                    nc.tensor.matmul(
                        kv, WKV16[:, kt, :], XT16[:, kt, sl],
                        start=(kt == 0), stop=(kt == KT - 1),
                    )
                kvs = rt.tile([128, SC], F16, tag="kvs")
                nc.scalar.copy(kvs[:], kv)
                return q0s, q1s, kvs

            def rope_part(qc, q0s, q1s, kvs):
                """RoPE combines + V transposes for chunk qc; returns the
                fp16 REIM pair tiles (rows [re_h;im_h] per head)."""
                sl = slice(qc * SC, (qc + 1) * SC)
                prec = qc == 0
                # v first: f16 transpose (PE) can start as soon as kvs lands
                nc.vector.tensor_copy(VT16[0:64, sl], kvs[64:128, :])
                for i4 in range(4):
                    kbg = 4 * qc + i4
                    sbi, ip = kbg // 2, kbg % 2
                    vp = scp_pool.tile([128, 64], F16, tag="sc", name="vp")
                    nc.tensor.transpose(
                        vp[:], VT16[0:64, kbg * 128 : (kbg + 1) * 128], IDENT16[:]
                    )
                    nc.vector.tensor_copy(VAUG8[:, sbi, ip, 0:64], vp[:])
                    if prec:
                        nc.vector.tensor_copy(VAUG16[:, i4, 0:64], vp[:])
                t1 = rt.tile([128, SC], F16, tag="t1")
                t2 = rt.tile([128, SC], F16, tag="t2")
                t3 = rt.tile([128, SC], F16, tag="t3")
                t4 = rt.tile([128, SC], F16, tag="t4")
                nc.vector.tensor_tensor(t1[:], q0s[:], COS[:, sl], OP.mult)
                nc.vector.tensor_tensor(t2[:], q1s[:], SIN[:, sl], OP.mult)
                nc.vector.tensor_tensor(t3[:], q0s[:], SIN[:, sl], OP.mult)
                nc.vector.tensor_tensor(t4[:], q1s[:], COS[:, sl], OP.mult)
                QF0 = qr.tile([128, SC], F16, tag="qf0")
                QF1 = qr.tile([128, SC], F16, tag="qf1")
                for h in range(4):
                    QF, half = (QF0, QF1)[h // 2], h % 2
                    rq = slice(32 * h, 32 * h + 32)
                    nc.vector.tensor_tensor(
                        QF[64 * half : 64 * half + 32, :],
                        t1[rq, :], t2[rq, :], OP.subtract,
                    )
                    nc.vector.tensor_tensor(
                        QF[64 * half + 32 : 64 * half + 64, :],
                        t3[rq, :], t4[rq, :], OP.add,
                    )

                # k rope into KA4 rows 0:64, duplicated to 64:128
                u1 = rt.tile([32, SC], F16, tag="u1")
                u2 = rt.tile([32, SC], F16, tag="u2")
                u3 = rt.tile([32, SC], F16, tag="u3")
                u4 = rt.tile([32, SC], F16, tag="u4")
                nc.vector.tensor_tensor(u1[:], kvs[0:32, :], COS[0:32, sl], OP.mult)
                nc.vector.tensor_tensor(u2[:], kvs[32:64, :], SIN[32:64, sl], OP.mult)
                nc.vector.tensor_tensor(KA4[0:32, sl], u1[:], u2[:], OP.subtract)
                nc.vector.tensor_tensor(u3[:], kvs[0:32, :], SIN[0:32, sl], OP.mult)
                nc.vector.tensor_tensor(u4[:], kvs[32:64, :], COS[32:64, sl], OP.mult)
                nc.vector.tensor_tensor(KA4[32:64, sl], u3[:], u4[:], OP.add)
                nc.sync.dma_start(KA4[64:128, sl], KA4[0:64, sl])

                return QF0, QF1

            def att_scores(qc, QF, sbi, p8s):
                """scores+exp+mask for superblock sbi of chunk qc; appends the
                p8 tile pair (one per head-pair) to p8s."""
                prec = qc == 0
                pr_tiles = [
                    pp.tile(
                        [128, 2, 2, SC], F16 if prec else F8,
                        tag=f"p16_{pr}" if prec else f"p8_{pr}",
                        name=f"p8_{pr}", bufs=6,
                    )
                    for pr in range(2)
                ]
                p8s.append(pr_tiles)
                for ip in range(2):
                    kb = 2 * sbi + ip
                    kre = kb - 4 * qc  # >= 0: diagonal block index
                    qlo = 128 * kre if kre > 0 else 0
                    ksl = slice(kb * 128, (kb + 1) * 128)
                    for pr in range(2):
                        scps = scp_pool.tile([128, 2, SC], F32, tag="sc")
                        for j in range(2):
                            rs = slice(64 * j, 64 * j + 64)
                            nc.tensor.matmul(
                                scps[:, j, qlo:SC],
                                KA4[rs, ksl],
                                QF[pr][rs, qlo:SC],
                                start=True, stop=True,
                                tile_position=(64 * j, 0),
                            )
                        p8 = pr_tiles[pr]
                        nc.scalar.activation(
                            p8[:, ip, :, qlo:SC], scps[:, :, qlo:SC],
                            AF.Exp, scale=0.125,
                        )
                        if kre >= 0:
                            if qlo > 0:
                                nc.vector.memset(p8[:, ip, :, 0:qlo], 0.0)
                            msl = slice(128 * kre, 128 * kre + 128)
                            nc.vector.tensor_tensor(
                                p8[:, ip, :, msl], p8[:, ip, :, msl],
                                MASK16[:] if prec else MASK2[:], OP.mult,
                            )

            def att_pv(qc, sbi, pr_tiles, ots):
                nkb = 4 * qc + 4
                nsb = nkb // 2
                prec = qc == 0
                for pr in range(2):
                    for j in range(2):
                        h = 2 * pr + j
                        if prec:
                            for ip in range(2):
                                kb = 2 * sbi + ip
                                nc.tensor.matmul(
                                    ots[h][0:65, :],
                                    VAUG16[:, kb, :],
                                    pr_tiles[pr][:, ip, j, :],
                                    start=(kb == 0), stop=(kb == nkb - 1),
                                )
                        else:
                            nc.tensor.matmul(
                                ots[h][:, :],
                                VAUG8[:, sbi, :, 0:128],
                                pr_tiles[pr][:, :, j, :],
                                start=(sbi == 0), stop=(sbi == nsb - 1),
                                perf_mode=DR,
                            )

            def norm_pre(qc, ots):
                """denominator gather + reciprocal; emit FIRST so the scalar
                copies sit ahead of the next chunk's exps in the ACT queue."""
                den_sb = npo.tile([128, SC], F32, tag="den_sb")
                nc.vector.memset(den_sb[:], 1.0)
                for h in range(4):
                    nc.vector.tensor_copy(
                        den_sb[32 * h : 32 * h + 1, :], ots[h][64:65, :]
                    )
                rec = npo.tile([128, SC], F16, tag="rec")
                with nc.allow_low_precision(reason="fp16 softmax denominators"):
                    nc.vector.reciprocal(rec[:], den_sb[:])
                return rec

            def norm_post(qc, ots, rec):
                qsl = slice(qc * SC, (qc + 1) * SC)
                for pr in range(2):
                    rbct = scp_pool.tile([128, 2, SC], F32, tag="sc", name=f"rbc{pr}")
                    rbc = rbct[:, 0, :]
                    nc.tensor.matmul(
                        rbc, SELP[:, pr, :], rec[:], start=True, stop=True
                    )
                    rbc_sb = npo.tile([128, SC], F16, tag="rbc_sb")
                    nc.vector.tensor_copy(rbc_sb[:], rbc)
                    att = (AT0, AT1)[pr]
                    for j in range(2):
                        rs = slice(64 * j, 64 * j + 64)
                        nc.vector.tensor_tensor(
                            att[rs, qsl], ots[2 * pr + j][0:64, :],
                            rbc_sb[rs, :], OP.mult,
                        )

            def oproj_one(qc, k):
                """k-th of the 8 out-proj blocks for chunk qc."""
                sb_i = 4 * qc + k // 2
                ec = k % 2
                ssl = slice(sb_i * 128, (sb_i + 1) * 128)
                esl = slice(ec * 512, (ec + 1) * 512)
                o_ps = scp_pool.tile([128, 2, SC], F32, tag="sc", name="o_ps")
                for t in range(2):
                    att = (AT0, AT1)[t]
                    nc.tensor.matmul(
                        o_ps[:, 0, :], att[:, ssl], WO[:, t, esl],
                        start=(t == 0), stop=(t == 1),
                    )
                ost = op_pool.tile([128, 512], F16, tag="ost")
                if k % 2 == 0:
                    nc.vector.tensor_copy(ost[:], o_ps[:, 0, :])
                    nc.sync.dma_start(out[ssl, esl], ost[:])
                else:
                    nc.scalar.copy(ost[:], o_ps[:, 0, :])
                    nc.scalar.dma_start(out[ssl, esl], ost[:])

            qfs = {}
            st0 = proj_mm(0)
            # HAM warm-up + startup-gap filler: dummy matmuls into a scratch
            # pv bank while DVE/scalar run chunk 0's rope chain
            warm = pvp.tile([128, SC], F32, tag="pv", name="warm")
            for _ in range(10):
                nc.tensor.matmul(
                    warm[:], WQ16[:, 0, 0:128], XT16[:, 0, 0:SC],
                    start=True, stop=True,
                )
            qfs[0] = rope_part(0, *st0)
            if NCH > 1:
                qfs[1] = rope_part(1, *proj_mm(1))
            prev = None  # (qc, ots) awaiting norm+oproj
            for qc in range(NCH):
                QF = qfs.pop(qc)
                nsb = (4 * qc + 4) // 2
                ots = [
                    pvp.tile([128, SC], F32, tag="pv", name=f"ot{qc}_{h}")
                    for h in range(4)
                ]
                p8s = []
                rec = norm_pre(*prev) if prev is not None else None
                # scores for the first superblocks keep PE+ACT busy while the
                # previous chunk's norm chain (DVE recip) runs
                pref = min(nsb, 4)
                for sbi in range(pref):
                    att_scores(qc, QF, sbi, p8s)
                if prev is not None:
                    norm_post(*prev, rec)
                for sbi in range(pref):
                    att_pv(qc, sbi, p8s[sbi], ots)
                # remaining superblocks, with the previous chunk's out-proj
                # spread between them so PE never idles through a MID window
                oleft = list(range(8)) if prev is not None else []
                rem = max(nsb - pref, 1)
                per = (len(oleft) + rem - 1) // rem if oleft else 0
                for sbi in range(pref, nsb):
                    att_scores(qc, QF, sbi, p8s)
                    for _ in range(per):
                        if oleft:
                            oproj_one(prev[0], oleft.pop(0))
                    att_pv(qc, sbi, p8s[sbi], ots)
                while oleft:
                    oproj_one(prev[0], oleft.pop(0))
                st = proj_mm(qc + 2) if qc + 2 < NCH else None
                prev = (qc, ots)
                if st is not None:
                    qfs[qc + 2] = rope_part(qc + 2, *st)
            norm_post(*prev, norm_pre(*prev))
            for k in range(8):
                oproj_one(prev[0], k)

    nc.compile()
    return nc


# host-side column permutations: all rope-even dims first, then all odds
_PERM256 = np.array(
    [64 * h + 2 * i for h in range(4) for i in range(32)]
    + [64 * h + 2 * i + 1 for h in range(4) for i in range(32)]
)
_PERM64 = np.array([2 * i for i in range(32)] + [2 * i + 1 for i in range(32)])

_cache = {}


def _fp8(a):
    return np.clip(a, -240.0, 240.0).astype(FP8T)


def make_in_maps(x, cos, sin, wq, wk, wv, wo, n_groups=4):
    S = x.shape[1]
    cos_r = np.ascontiguousarray(np.tile(cos.T, (4, 1)), dtype=np.float16)
    sin_r = np.ascontiguousarray(np.tile(sin.T, (4, 1)), dtype=np.float16)
    # scores_T[k, q] valid iff q >= k within the diagonal 128-block
    pidx, qidx = np.meshgrid(np.arange(128), np.arange(128), indexing="ij")
    tri = (qidx >= pidx).astype(np.float32)
    mask2 = _fp8(np.broadcast_to(tri[:, None, :], (128, 2, 128)).copy())
    mask16 = np.ascontiguousarray(
        np.broadcast_to(tri[:, None, :], (128, 2, 128)).astype(np.float16)
    )
    ident16 = np.eye(64, dtype=np.float16)
    selp = np.zeros((128, 2, 128), dtype=np.float16)
    selp[0, 0, 0:64] = 1.0
    selp[32, 0, 64:128] = 1.0
    selp[64, 1, 0:64] = 1.0
    selp[96, 1, 64:128] = 1.0
    xt16s = []
    for b in range(x.shape[0]):
        xT = x[b].T.astype(np.float16)  # [1024, S]
        xt16s.append(
            np.ascontiguousarray(xT.reshape(KT, 128, S).transpose(1, 0, 2))
        )
    in_maps = []
    for c in range(x.shape[0] * n_groups):
        b, g = divmod(c, n_groups)
        wq_c = wq[:, 256 * g + _PERM256].astype(np.float16)
        wk_c = wk[:, 64 * g + _PERM64]
        wv_c = wv[:, 64 * g : 64 * (g + 1)]
        wkv_c = np.concatenate([wk_c, wv_c], axis=1).astype(np.float16)
        wo_c = np.ascontiguousarray(
            wo[256 * g : 256 * (g + 1), :].astype(np.float16)
            .reshape(2, 128, DIM).transpose(1, 0, 2)
        )
        in_maps.append(
            {
                "xt16": xt16s[b],
                "wq16": np.ascontiguousarray(
                    wq_c.reshape(KT, 128, 256).transpose(1, 0, 2)
                ),
                "wkv16": np.ascontiguousarray(
                    wkv_c.reshape(KT, 128, 128).transpose(1, 0, 2)
                ),
                "wo": wo_c,
                "cosr": cos_r,
                "sinr": sin_r,
                "mask2": mask2,
                "mask16": mask16,
                "ident16": ident16,
                "selp": selp,
            }
        )
    return in_maps


def kernel(x, cos, sin, mask, wq, wk, wv, wo):
    x = np.asarray(x, dtype=np.float32)
    cos = np.asarray(cos, dtype=np.float32)
    sin = np.asarray(sin, dtype=np.float32)
    wq = np.asarray(wq, dtype=np.float32)
    wk = np.asarray(wk, dtype=np.float32)
    wv = np.asarray(wv, dtype=np.float32)
    wo = np.asarray(wo, dtype=np.float32)

    if "nc" not in _cache:
        _cache["nc"] = build_nc(S=x.shape[1], n_cores=8)
    nc = _cache["nc"]
    in_maps = make_in_maps(x, cos, sin, wq, wk, wv, wo)
    res = run_bass_kernel_spmd(nc, in_maps, list(range(8)))
    _cache["last"] = res
    outs = [np.asarray(r["out"], dtype=np.float32) for r in res.results]
    final = np.stack(
        [outs[0] + outs[1] + outs[2] + outs[3], outs[4] + outs[5] + outs[6] + outs[7]],
        axis=0,
    )
    return final.astype(np.float32)


# revision 43
# speedup vs baseline: 1.2488x; 1.0279x over previous
"""GQA attention (RoPE + causal softmax + out-proj) on 8 TRN2 cores.

Sharding: one core per (batch b, kv-head-group g): 2 batches x 4 kv groups = 8
cores. Each core computes its group's 4 query heads end to end, including the
partial output projection through its 256 rows of wo; the host sums the 4
partial projections per batch (fp16 partials, f32 host sum).

v3: measured on HW, fp8 DoubleRow matmuls trip the PE's half-rate power
throttle when used broadly (util limit 53%, DR avg 589ns vs fp16 483ns), so
fp16 is used for all high-volume matmuls (projections, scores, out-proj,
baseline-style tile_position packing) and fp8 DoubleRow is kept ONLY where it
halves the instruction count: PV over 256-row k superblocks, with the exp'd
probabilities written by ACT directly into the DoubleRow rhs layout
p8 [128p(k), 2(kb parity), 2(head of pair), 512] -- zero data movement.
Structural wins vs the fp16 baseline:
  - no mask matmuls: causality via partial-width exp + gpsimd 0/1-triangle
    multiply + gpsimd memset of the dead region.
  - PV lhsT carries an all-ones column, so each head's [65+, 512] psum tile
    accumulates its softmax denominator in row 64 (DoubleRow dst must start
    at partition 0); one reciprocal per chunk + SELP broadcast matmuls.
  - rows 0:512 (chunk 0) run fully in fp16 (P and V): tiny softmax support
    there means fp8 P/V noise does not average out and would blow the
    max-norm error; later rows have large support and tolerate fp8.
  - software pipeline: projection matmuls for chunk c+2 are emitted between
    attention(c) and the norm chain so PE has work while DVE runs the
    reciprocal; rope/transposes follow after.
  - fp16 output tensor (host sums partials in f32): halves the output DMA.
"""

import os
import sys
import types

import numpy as np
import ml_dtypes


def _ensure_axon_hooks_shim():
    """The agent image's antenv package lacks the axon_hooks submodule that
    concourse's trace path imports; install a stub so trace requests degrade
    to no-trace instead of crashing (a real hook can be set into the stub)."""
    try:
        import antenv.axon_hooks  # noqa: F401

        return
    except ImportError:
        pass
    try:
        import antenv
    except ImportError:
        return
    mod = types.ModuleType("antenv.axon_hooks")
    mod._AXON_NTFF_PROFILE_HOOK = None

    def get_axon_ntff_profile_hook():
        return mod._AXON_NTFF_PROFILE_HOOK

    def set_axon_ntff_profile_hook(hook):
        mod._AXON_NTFF_PROFILE_HOOK = hook

    mod.get_axon_ntff_profile_hook = get_axon_ntff_profile_hook
    mod.set_axon_ntff_profile_hook = set_axon_ntff_profile_hook
    sys.modules["antenv.axon_hooks"] = mod
    antenv.axon_hooks = mod


_ensure_axon_hooks_shim()

import concourse.bass as bass
import concourse.bacc as bacc
import concourse.mybir as mybir
import concourse.tile as tile
from concourse.bass_utils import run_bass_kernel_spmd

F32 = mybir.dt.float32
F16 = mybir.dt.float16
F8 = mybir.dt.float8e4
AF = mybir.ActivationFunctionType
OP = mybir.AluOpType
DR = mybir.MatmulPerfMode.DoubleRow
FP8T = ml_dtypes.float8_e4m3

B, DIM = 2, 1024
NH, NKV, HD = 16, 4, 64
S_FULL = 2048
SC = 512  # q chunk width
KT = DIM // 128  # 8 k-tiles over the model dim


def build_nc(S=S_FULL, n_cores=8):
    NCH = S // SC
    NSB = S // 256  # k superblocks (256 rows each)

    nc = bacc.Bacc(
        "TRN2", target_bir_lowering=False, debug=False, num_devices=n_cores
    )
    xt16d = nc.dram_tensor("xt16", [128, KT, SC], F16, kind="ExternalInput").ap()
    x8d = nc.dram_tensor("x8", [128, KT // 2, 2, S], F8, kind="ExternalInput").ap()
    wq8d = nc.dram_tensor("wq8", [128, 2, KT // 2, 256], F8, kind="ExternalInput").ap()
    wkv8d = nc.dram_tensor("wkv8", [128, 2, KT // 2, 128], F8, kind="ExternalInput").ap()
    wq16d = nc.dram_tensor("wq16", [128, KT, 256], F16, kind="ExternalInput").ap()
    wkv16d = nc.dram_tensor("wkv16", [128, KT, 128], F16, kind="ExternalInput").ap()
    wod = nc.dram_tensor("wo", [128, 2, DIM], F16, kind="ExternalInput").ap()
    cosr = nc.dram_tensor("cosr", [128, S], F16, kind="ExternalInput").ap()
    sinr = nc.dram_tensor("sinr", [128, S], F16, kind="ExternalInput").ap()
    mask2d = nc.dram_tensor("mask2", [128, 2, 128], F8, kind="ExternalInput").ap()
    mask16d = nc.dram_tensor("mask16", [128, 2, 128], F16, kind="ExternalInput").ap()
    ident16d = nc.dram_tensor("ident16", [64, 64], F16, kind="ExternalInput").ap()
    selpd = nc.dram_tensor("selp", [128, 2, 128], F16, kind="ExternalInput").ap()
    out = nc.dram_tensor("out", [S, DIM], F16, kind="ExternalOutput").ap()

    with tile.TileContext(nc) as tc:
        with (
            tc.tile_pool(name="const", bufs=1) as cp,
            tc.tile_pool(name="sc", bufs=2, space="PSUM") as scp_pool,
            tc.tile_pool(name="pv", bufs=4, space="PSUM") as pvp,
            tc.tile_pool(name="rt", bufs=2) as rt,
            tc.tile_pool(name="qr", bufs=3) as qr,
            tc.tile_pool(name="pp", bufs=2) as pp,
            tc.tile_pool(name="np_", bufs=2) as npo,
            tc.tile_pool(name="op", bufs=3) as op_pool,
        ):
            COS = cp.tile([128, S], F16, tag="COS")
            SIN = cp.tile([128, S], F16, tag="SIN")
            WO = cp.tile([128, 2, DIM], F16, tag="WO")
            MASK2 = cp.tile([128, 2, 128], F8, tag="MASK2")
            MASK16 = cp.tile([128, 2, 128], F16, tag="MASK16")
            SELP = cp.tile([128, 2, 128], F16, tag="SELP")
            XT16 = cp.tile([128, KT, SC], F16, tag="XT16")
            X8 = cp.tile([128, KT // 2, 2, S], F8, tag="X8")
            WQ8 = cp.tile([128, 2, KT // 2, 256], F8, tag="WQ8")
            WKV8 = cp.tile([128, 2, KT // 2, 128], F8, tag="WKV8")
            WQ16 = cp.tile([128, KT, 256], F16, tag="WQ16")
            WKV16 = cp.tile([128, KT, 128], F16, tag="WKV16")
            IDENT16 = cp.tile([64, 64], F16, tag="IDENT16")
            KA4 = cp.tile([128, S], F16, tag="KA4")  # [kre;kim] x2 row-dup
            VT16 = cp.tile([64, S], F16, tag="VT16")
            VAUG8 = cp.tile([128, NSB, 2, 128], F8, tag="VAUG8")
            VAUG16 = cp.tile([128, 4, 65], F16, tag="VAUG16")
            AT0 = cp.tile([128, S], F16, tag="AT0")
            AT1 = cp.tile([128, S], F16, tag="AT1")

            # startup DMAs: each dma_start costs ~0.6us of ISSUE time on
            # its queue, so spread across sync/scalar/vector queues and order
            # by first use (proj(0) needs WQ16 + XT16 head immediately)
            nc.scalar.dma_start(WQ16[:], wq16d)
            nc.scalar.dma_start(WKV16[:], wkv16d)
            nc.scalar.dma_start(MASK16[:], mask16d)
            nc.scalar.dma_start(MASK2[:], mask2d)
            nc.scalar.dma_start(COS[:], cosr)
            nc.scalar.dma_start(SIN[:], sinr)
            nc.scalar.dma_start(IDENT16[:], ident16d)
            nc.scalar.dma_start(SELP[:], selpd)
            for kt in range(KT):
                nc.sync.dma_start(XT16[:, kt, :], xt16d[:, kt, :])
            nc.scalar.dma_start(WQ8[:], wq8d)
            nc.scalar.dma_start(WKV8[:], wkv8d)
            if S > SC:
                for ktp in range(KT // 2):
                    nc.sync.dma_start(X8[:, ktp, :, SC:], x8d[:, ktp, :, SC:])
            nc.sync.dma_start(WO[:], wod)
            # ones column for the PV denominators (DoubleRow dst must start
            # at partition 0, so dens ride along as output row 64)
            nc.vector.memset(VAUG8[:, :, :, 64:128], 0.0)
            nc.vector.memset(VAUG8[:, :, :, 64:65], 1.0)
            nc.vector.memset(VAUG16[:, :, 64:65], 1.0)

            def proj_mm(qc):
                """Projection matmuls + psum->sbuf copies for chunk qc.
                Emitted early so PE has work while the norm chain runs."""
                sl = slice(qc * SC, (qc + 1) * SC)
                prec = qc == 0
                KP = KT // 2
                q0t = scp_pool.tile([128, 2, SC], F32, tag="sc", name="q0t")
                q0 = q0t[:, 0, :]
                if prec:
                    for kt in range(KT):
                        nc.tensor.matmul(
                            q0, WQ16[:, kt, 0:128], XT16[:, kt, :],
                            start=(kt == 0), stop=(kt == KT - 1),
                        )
                else:
                    for kp in range(KP):
                        nc.tensor.matmul(
                            q0, WQ8[:, :, kp, 0:128], X8[:, kp, :, sl],
                            start=(kp == 0), stop=(kp == KP - 1), perf_mode=DR,
                        )
                q0s = rt.tile([128, SC], F16, tag="q0s")
                nc.scalar.copy(q0s[:], q0)
                q1t = scp_pool.tile([128, 2, SC], F32, tag="sc", name="q1t")
                q1 = q1t[:, 0, :]
                if prec:
                    for kt in range(KT):
                        nc.tensor.matmul(
                            q1, WQ16[:, kt, 128:256], XT16[:, kt, :],
                            start=(kt == 0), stop=(kt == KT - 1),
                        )
                else:
                    for kp in range(KP):
                        nc.tensor.matmul(
                            q1, WQ8[:, :, kp, 128:256], X8[:, kp, :, sl],
                            start=(kp == 0), stop=(kp == KP - 1), perf_mode=DR,
                        )
                q1s = rt.tile([128, SC], F16, tag="q1s")
                nc.scalar.copy(q1s[:], q1)
                kvt = scp_pool.tile([128, 2, SC], F32, tag="sc", name="kvt")
                kv = kvt[:, 0, :]
                if prec:
                    for kt in range(KT):
                        nc.tensor.matmul(
                            kv, WKV16[:, kt, :], XT16[:, kt, :],
                            start=(kt == 0), stop=(kt == KT - 1),
                        )
                else:
                    for kp in range(KP):
                        nc.tensor.matmul(
                            kv, WKV8[:, :, kp, :], X8[:, kp, :, sl],
                            start=(kp == 0), stop=(kp == KP - 1), perf_mode=DR,
                        )
                kvs = rt.tile([128, SC], F16, tag="kvs")
                nc.scalar.copy(kvs[:], kv)
                return q0s, q1s, kvs

            def rope_part(qc, q0s, q1s, kvs):
                """RoPE combines + V transposes for chunk qc; returns the
                fp16 REIM pair tiles (rows [re_h;im_h] per head)."""
                sl = slice(qc * SC, (qc + 1) * SC)
                prec = qc == 0
                # v first: f16 transpose (PE) can start as soon as kvs lands
                nc.vector.tensor_copy(VT16[0:64, sl], kvs[64:128, :])
                for i4 in range(4):
                    kbg = 4 * qc + i4
                    sbi, ip = kbg // 2, kbg % 2
                    vp = scp_pool.tile([128, 64], F16, tag="sc", name="vp")
                    nc.tensor.transpose(
                        vp[:], VT16[0:64, kbg * 128 : (kbg + 1) * 128], IDENT16[:]
                    )
                    nc.vector.tensor_copy(VAUG8[:, sbi, ip, 0:64], vp[:])
                    if prec:
                        nc.vector.tensor_copy(VAUG16[:, i4, 0:64], vp[:])
                t1 = rt.tile([128, SC], F16, tag="t1")
                t2 = rt.tile([128, SC], F16, tag="t2")
                t3 = rt.tile([128, SC], F16, tag="t3")
                t4 = rt.tile([128, SC], F16, tag="t4")
                nc.vector.tensor_tensor(t1[:], q0s[:], COS[:, sl], OP.mult)
                nc.vector.tensor_tensor(t2[:], q1s[:], SIN[:, sl], OP.mult)
                nc.vector.tensor_tensor(t3[:], q0s[:], SIN[:, sl], OP.mult)
                nc.vector.tensor_tensor(t4[:], q1s[:], COS[:, sl], OP.mult)
                QF0 = qr.tile([128, SC], F16, tag="qf0")
                QF1 = qr.tile([128, SC], F16, tag="qf1")
                for h in range(4):
                    QF, half = (QF0, QF1)[h // 2], h % 2
                    rq = slice(32 * h, 32 * h + 32)
                    nc.vector.tensor_tensor(
                        QF[64 * half : 64 * half + 32, :],
                        t1[rq, :], t2[rq, :], OP.subtract,
                    )
                    nc.vector.tensor_tensor(
                        QF[64 * half + 32 : 64 * half + 64, :],
                        t3[rq, :], t4[rq, :], OP.add,
                    )

                # k rope into KA4 rows 0:64, duplicated to 64:128
                u1 = rt.tile([32, SC], F16, tag="u1")
                u2 = rt.tile([32, SC], F16, tag="u2")
                u3 = rt.tile([32, SC], F16, tag="u3")
                u4 = rt.tile([32, SC], F16, tag="u4")
                nc.vector.tensor_tensor(u1[:], kvs[0:32, :], COS[0:32, sl], OP.mult)
                nc.vector.tensor_tensor(u2[:], kvs[32:64, :], SIN[32:64, sl], OP.mult)
                nc.vector.tensor_tensor(KA4[0:32, sl], u1[:], u2[:], OP.subtract)
                nc.vector.tensor_tensor(u3[:], kvs[0:32, :], SIN[0:32, sl], OP.mult)
                nc.vector.tensor_tensor(u4[:], kvs[32:64, :], COS[32:64, sl], OP.mult)
                nc.vector.tensor_tensor(KA4[32:64, sl], u3[:], u4[:], OP.add)
                nc.sync.dma_start(KA4[64:128, sl], KA4[0:64, sl])

                return QF0, QF1

            def att_scores(qc, QF, sbi, p8s):
                """scores+exp+mask for superblock sbi of chunk qc; appends the
                p8 tile pair (one per head-pair) to p8s."""
                prec = qc == 0
                pr_tiles = [
                    pp.tile(
                        [128, 2, 2, SC], F16 if prec else F8,
                        tag=f"p16_{pr}" if prec else f"p8_{pr}",
                        name=f"p8_{pr}", bufs=6,
                    )
                    for pr in range(2)
                ]
                p8s.append(pr_tiles)
                for ip in range(2):
                    kb = 2 * sbi + ip
                    kre = kb - 4 * qc  # >= 0: diagonal block index
                    qlo = 128 * kre if kre > 0 else 0
                    ksl = slice(kb * 128, (kb + 1) * 128)
                    for pr in range(2):
                        scps = scp_pool.tile([128, 2, SC], F32, tag="sc")
                        for j in range(2):
                            rs = slice(64 * j, 64 * j + 64)
                            nc.tensor.matmul(
                                scps[:, j, qlo:SC],
                                KA4[rs, ksl],
                                QF[pr][rs, qlo:SC],
                                start=True, stop=True,
                                tile_position=(64 * j, 0),
                            )
                        p8 = pr_tiles[pr]
                        nc.scalar.activation(
                            p8[:, ip, :, qlo:SC], scps[:, :, qlo:SC],
                            AF.Exp, scale=0.125,
                        )
                        if kre >= 0:
                            if qlo > 0:
                                nc.vector.memset(p8[:, ip, :, 0:qlo], 0.0)
                            msl = slice(128 * kre, 128 * kre + 128)
                            nc.vector.tensor_tensor(
                                p8[:, ip, :, msl], p8[:, ip, :, msl],
                                MASK16[:] if prec else MASK2[:], OP.mult,
                            )

            def att_pv(qc, sbi, pr_tiles, ots):
                nkb = 4 * qc + 4
                nsb = nkb // 2
                prec = qc == 0
                for pr in range(2):
                    for j in range(2):
                        h = 2 * pr + j
                        if prec:
                            for ip in range(2):
                                kb = 2 * sbi + ip
                                nc.tensor.matmul(
                                    ots[h][0:65, :],
                                    VAUG16[:, kb, :],
                                    pr_tiles[pr][:, ip, j, :],
                                    start=(kb == 0), stop=(kb == nkb - 1),
                                )
                        else:
                            nc.tensor.matmul(
                                ots[h][:, :],
                                VAUG8[:, sbi, :, 0:128],
                                pr_tiles[pr][:, :, j, :],
                                start=(sbi == 0), stop=(sbi == nsb - 1),
                                perf_mode=DR,
                            )

            def norm_pre(qc, ots):
                """denominator gather + reciprocal; emit FIRST so the scalar
                copies sit ahead of the next chunk's exps in the ACT queue."""
                den_sb = npo.tile([128, SC], F32, tag="den_sb")
                nc.vector.memset(den_sb[:], 1.0)
                for h in range(4):
                    nc.vector.tensor_copy(
                        den_sb[32 * h : 32 * h + 1, :], ots[h][64:65, :]
                    )
                rec = npo.tile([128, SC], F16, tag="rec")
                with nc.allow_low_precision(reason="fp16 softmax denominators"):
                    nc.vector.reciprocal(rec[:], den_sb[:])
                return rec

            def norm_post(qc, ots, rec):
                qsl = slice(qc * SC, (qc + 1) * SC)
                for pr in range(2):
                    rbct = scp_pool.tile([128, 2, SC], F32, tag="sc", name=f"rbc{pr}")
                    rbc = rbct[:, 0, :]
                    nc.tensor.matmul(
                        rbc, SELP[:, pr, :], rec[:], start=True, stop=True
                    )
                    rbc_sb = npo.tile([128, SC], F16, tag="rbc_sb")
                    nc.vector.tensor_copy(rbc_sb[:], rbc)
                    att = (AT0, AT1)[pr]
                    for j in range(2):
                        rs = slice(64 * j, 64 * j + 64)
                        nc.vector.tensor_tensor(
                            att[rs, qsl], ots[2 * pr + j][0:64, :],
                            rbc_sb[rs, :], OP.mult,
                        )

            def oproj_one(qc, k):
                """k-th of the 8 out-proj blocks for chunk qc."""
                sb_i = 4 * qc + k // 2
                ec = k % 2
                ssl = slice(sb_i * 128, (sb_i + 1) * 128)
                esl = slice(ec * 512, (ec + 1) * 512)
                o_ps = scp_pool.tile([128, 2, SC], F32, tag="sc", name="o_ps")
                for t in range(2):
                    att = (AT0, AT1)[t]
                    nc.tensor.matmul(
                        o_ps[:, 0, :], att[:, ssl], WO[:, t, esl],
                        start=(t == 0), stop=(t == 1),
                    )
                ost = op_pool.tile([128, 512], F16, tag="ost")
                if k % 2 == 0:
                    nc.vector.tensor_copy(ost[:], o_ps[:, 0, :])
                    nc.sync.dma_start(out[ssl, esl], ost[:])
                else:
                    nc.scalar.copy(ost[:], o_ps[:, 0, :])
                    nc.scalar.dma_start(out[ssl, esl], ost[:])

            qfs = {}
            st0 = proj_mm(0)
            # HAM warm-up + startup-gap filler: dummy matmuls into a scratch
            # pv bank while DVE/scalar run chunk 0's rope chain
            warm = pvp.tile([128, SC], F32, tag="pv", name="warm")
            for _ in range(10):
                nc.tensor.matmul(
                    warm[:], WQ16[:, 0, 0:128], XT16[:, 0, 0:SC],
                    start=True, stop=True,
                )
            qfs[0] = rope_part(0, *st0)
            if NCH > 1:
                qfs[1] = rope_part(1, *proj_mm(1))
            prev = None  # (qc, ots) awaiting norm+oproj
            for qc in range(NCH):
                QF = qfs.pop(qc)
                nsb = (4 * qc + 4) // 2
                ots = [
                    pvp.tile([128, SC], F32, tag="pv", name=f"ot{qc}_{h}")
                    for h in range(4)
                ]
                p8s = []
                rec = norm_pre(*prev) if prev is not None else None
                # scores for the first superblocks keep PE+ACT busy while the
                # previous chunk's norm chain (DVE recip) runs
                pref = min(nsb, 4)
                for sbi in range(pref):
                    att_scores(qc, QF, sbi, p8s)
                if prev is not None:
                    norm_post(*prev, rec)
                for sbi in range(pref):
                    att_pv(qc, sbi, p8s[sbi], ots)
                # remaining superblocks, with the previous chunk's out-proj
                # spread between them so PE never idles through a MID window
                oleft = list(range(8)) if prev is not None else []
                rem = max(nsb - pref, 1)
                per = (len(oleft) + rem - 1) // rem if oleft else 0
                for sbi in range(pref, nsb):
                    att_scores(qc, QF, sbi, p8s)
                    for _ in range(per):
                        if oleft:
                            oproj_one(prev[0], oleft.pop(0))
                    att_pv(qc, sbi, p8s[sbi], ots)
                while oleft:
                    oproj_one(prev[0], oleft.pop(0))
                st = proj_mm(qc + 2) if qc + 2 < NCH else None
                prev = (qc, ots)
                if st is not None:
                    qfs[qc + 2] = rope_part(qc + 2, *st)
            norm_post(*prev, norm_pre(*prev))
            for k in range(8):
                oproj_one(prev[0], k)

    nc.compile()
    return nc


# host-side column permutations: all rope-even dims first, then all odds
_PERM256 = np.array(
    [64 * h + 2 * i for h in range(4) for i in range(32)]
    + [64 * h + 2 * i + 1 for h in range(4) for i in range(32)]
)
_PERM64 = np.array([2 * i for i in range(32)] + [2 * i + 1 for i in range(32)])

_cache = {}


def _fp8(a):
    return np.clip(a, -240.0, 240.0).astype(FP8T)


def make_in_maps(x, cos, sin, wq, wk, wv, wo, n_groups=4):
    S = x.shape[1]
    cos_r = np.ascontiguousarray(np.tile(cos.T, (4, 1)), dtype=np.float16)
    sin_r = np.ascontiguousarray(np.tile(sin.T, (4, 1)), dtype=np.float16)
    # scores_T[k, q] valid iff q >= k within the diagonal 128-block
    pidx, qidx = np.meshgrid(np.arange(128), np.arange(128), indexing="ij")
    tri = (qidx >= pidx).astype(np.float32)
    mask2 = _fp8(np.broadcast_to(tri[:, None, :], (128, 2, 128)).copy())
    mask16 = np.ascontiguousarray(
        np.broadcast_to(tri[:, None, :], (128, 2, 128)).astype(np.float16)
    )
    ident16 = np.eye(64, dtype=np.float16)
    selp = np.zeros((128, 2, 128), dtype=np.float16)
    selp[0, 0, 0:64] = 1.0
    selp[32, 0, 64:128] = 1.0
    selp[64, 1, 0:64] = 1.0
    selp[96, 1, 64:128] = 1.0
    xt16s, x8s = [], []
    for b in range(x.shape[0]):
        xT32 = x[b].T.astype(np.float32)  # [1024, S]
        xT = xT32.astype(np.float16)
        xt16s.append(
            np.ascontiguousarray(
                xT[:, 0:SC].reshape(KT, 128, SC).transpose(1, 0, 2)
            )
        )
        x8s.append(
            np.ascontiguousarray(
                _fp8(xT32).reshape(KT // 2, 2, 128, S).transpose(2, 0, 1, 3)
            )
        )
    in_maps = []
    for c in range(x.shape[0] * n_groups):
        b, g = divmod(c, n_groups)
        wq_c32 = wq[:, 256 * g + _PERM256].astype(np.float32)
        wq_c = wq_c32.astype(np.float16)
        wq8 = np.ascontiguousarray(
            _fp8(wq_c32).reshape(KT // 2, 2, 128, 256).transpose(2, 1, 0, 3)
        )
        wk_c = wk[:, 64 * g + _PERM64]
        wv_c = wv[:, 64 * g : 64 * (g + 1)]
        wkv_c32 = np.concatenate([wk_c, wv_c], axis=1).astype(np.float32)
        wkv_c = wkv_c32.astype(np.float16)
        wkv8 = np.ascontiguousarray(
            _fp8(wkv_c32).reshape(KT // 2, 2, 128, 128).transpose(2, 1, 0, 3)
        )
        wo_c = np.ascontiguousarray(
            wo[256 * g : 256 * (g + 1), :].astype(np.float16)
            .reshape(2, 128, DIM).transpose(1, 0, 2)
        )
        in_maps.append(
            {
                "xt16": xt16s[b],
                "x8": x8s[b],
                "wq8": wq8,
                "wkv8": wkv8,
                "wq16": np.ascontiguousarray(
                    wq_c.reshape(KT, 128, 256).transpose(1, 0, 2)
                ),
                "wkv16": np.ascontiguousarray(
                    wkv_c.reshape(KT, 128, 128).transpose(1, 0, 2)
                ),
                "wo": wo_c,
                "cosr": cos_r,
                "sinr": sin_r,
                "mask2": mask2,
                "mask16": mask16,
                "ident16": ident16,
                "selp": selp,
            }
        )
    return in_maps


def kernel(x, cos, sin, mask, wq, wk, wv, wo):
    x = np.asarray(x, dtype=np.float32)
    cos = np.asarray(cos, dtype=np.float32)
    sin = np.asarray(sin, dtype=np.float32)
    wq = np.asarray(wq, dtype=np.float32)
    wk = np.asarray(wk, dtype=np.float32)
    wv = np.asarray(wv, dtype=np.float32)
    wo = np.asarray(wo, dtype=np.float32)

    if "nc" not in _cache:
        _cache["nc"] = build_nc(S=x.shape[1], n_cores=8)
    nc = _cache["nc"]
    in_maps = make_in_maps(x, cos, sin, wq, wk, wv, wo)
    res = run_bass_kernel_spmd(nc, in_maps, list(range(8)))
    _cache["last"] = res
    outs = [np.asarray(r["out"], dtype=np.float32) for r in res.results]
    final = np.stack(
        [outs[0] + outs[1] + outs[2] + outs[3], outs[4] + outs[5] + outs[6] + outs[7]],
        axis=0,
    )
    return final.astype(np.float32)
